# revision 34
# baseline (speedup 1.0000x reference)
"""MoE gate (softmax + bias-adjusted top-8 routing) Trainium2 Bass kernel.

Full inputs in, full outputs out. Token dim (B*S = 32768) is sharded 8 ways
across NeuronCores; the tiny gate weight [E,H] and expert biases [E] are
replicated. Each core computes logits = x @ W^T, fp32 softmax, top-8 by
bias-adjusted score, and normalized top-8 weights entirely on device.

Per-core layout choices:
  - x shard is shipped host-transposed as x^T [H, T_core] so the contraction
    dim (H) lands on SBUF partitions with fully contiguous DMA loads.
  - logits [128t, 64e] accumulate in PSUM over 16 H-chunks with the x^T
    chunk as the stationary PE operand and W^T streaming (plain fp32 for
    exact-precision routing decisions).
  - top-8 via DVE max8/max_index; bias[idx] gather via one-hot arithmetic.
"""

import os
import sys
from contextlib import ExitStack

import numpy as np

sys.path.insert(0, "/opt/trn_rl_repo")

import concourse.bacc as bacc
import concourse.bass as bass
import concourse.mybir as mybir
import concourse.tile as tile

B, S, H, E, K = 8, 4096, 2048, 64, 8
N_CORES = 8
T = B * S
T_CORE = T // N_CORES  # 4096 tokens per core
TG = 512               # tokens per group
NSUB = TG // 128       # 128-token subtiles per group
KH = H // 128          # contraction chunks

f32 = mybir.dt.float32
i32 = mybir.dt.int32
u16 = mybir.dt.uint16
Alu = mybir.AluOpType
Act = mybir.ActivationFunctionType
Ax = mybir.AxisListType


def build_nc(t_core=T_CORE):
    G = t_core // TG
    nc = bacc.Bacc("TRN2", target_bir_lowering=False, debug=False,
                   enable_asserts=False)
    xt = nc.dram_tensor("xt", [H, t_core], f32, kind="ExternalInput").ap()
    wt = nc.dram_tensor("wt", [H, E], f32, kind="ExternalInput").ap()
    eb = nc.dram_tensor("eb", [E], f32, kind="ExternalInput").ap()
    # Outputs in [128p, G, NSUB, K] layout (token = (g*NSUB+j)*128 + p) so the
    # store DMA is fully contiguous; host reorders the tiny result.
    idx_out = nc.dram_tensor("idx_out", [128, G, NSUB, K], i32,
                             kind="ExternalOutput").ap()
    w_out = nc.dram_tensor("w_out", [128, G, NSUB, K], f32,
                           kind="ExternalOutput").ap()

    with tile.TileContext(nc) as tc:
        with ExitStack() as ctx:
            _emit(ctx, tc, nc, xt, wt, eb, idx_out, w_out, G)
    nc.compile()
    return nc


def _emit(ctx, tc, nc, xt, wt, eb, idx_out, w_out, G):
    const = ctx.enter_context(tc.tile_pool(name="const", bufs=1))
    xtp = ctx.enter_context(tc.tile_pool(name="xtp", bufs=4))
    psB = ctx.enter_context(tc.tile_pool(name="psB", bufs=3, space="PSUM"))
    wk = ctx.enter_context(tc.tile_pool(name="wk", bufs=3))
    ohp = ctx.enter_context(tc.tile_pool(name="ohp", bufs=3))
    outp = ctx.enter_context(tc.tile_pool(name="outp", bufs=3))

    # Constants: W^T chunks (streamed matmul operand), broadcast biases,
    # expert-id iota row.
    wt_sb = const.tile([128, KH, E], f32)
    nc.sync.dma_start(out=wt_sb,
                      in_=wt.rearrange("(k p) e -> p k e", p=128))
    bias_sb = const.tile([128, E], f32)
    nc.gpsimd.dma_start(out=bias_sb, in_=eb.unsqueeze(0).broadcast_to((128, E)))
    iota = const.tile([128, E], f32)
    nc.gpsimd.iota(iota, pattern=[[1, E]], base=0, channel_multiplier=0,
                   allow_small_or_imprecise_dtypes=True)

    # PE matmuls lower to LDW+MM structs that can carry only ONE sync wait.
    # Consume the W^T DMA dep with a single-wait PE warmup op so loop matmuls
    # each need at most one (their x-tile DMA).
    scr = ctx.enter_context(tc.tile_pool(name="scr", bufs=1, space="PSUM"))
    warm_m = scr.tile([64, 64], f32, tag="warm_m")
    nc.tensor.matmul(warm_m, lhsT=wt_sb[:, 0, :],
                     rhs=wt_sb[:, 0, :], start=True, stop=True)
    # likewise pre-consume the bias-broadcast DMA on DVE and ACT
    warm_v = const.tile([128, 1], f32, tag="warm_v")
    nc.vector.tensor_copy(warm_v, bias_sb[:, 0:1])
    warm_a = const.tile([128, 1], f32, tag="warm_a")
    nc.scalar.copy(warm_a, bias_sb[:, 0:1])
    # Pool instructions aren't ordered across Q7 cores: consume the iota
    # production tick on Pool's own sem too
    warm_p2 = const.tile([128, 1], f32, tag="warm_p2")
    nc.gpsimd.tensor_copy(warm_p2, iota[:, 0:1])

    xt_r = xt.rearrange("(k p) (g t) -> g p k t", p=128, t=TG)

    # gather helpers kept 3D (walrus limits STT/TT inputs to 2-3 dims)
    bias_b3 = bias_sb.unsqueeze(1).to_broadcast([128, NSUB * K, E])
    iota_b3 = iota.unsqueeze(1).to_broadcast([128, NSUB * K, E])

    KQ = 4               # x-tile DMA split: KH/KQ chunks per sub-DMA
    KHQ = KH // KQ

    for g in range(G):
        # ---- load x^T group as KQ separate chunk tiles [128h, KHQ, 512t]
        # so PE can start accumulating after the first 1MB lands
        xgs = []
        for q in range(KQ):
            xq = xtp.tile([128, KHQ, TG], f32, tag=f"xg{q}")
            nc.sync.dma_start(out=xq, in_=xt_r[g][:, q * KHQ:(q + 1) * KHQ, :])
            xgs.append(xq)

        # ---- logits [128t, NSUB, 64e]: x^T chunk stationary, W^T streams.
        # Accumulation groups must stay contiguous per PSUM region (HW
        # verified: interleaving corrupts results), so j outer / k inner;
        # j=0 still starts as soon as the first chunk DMA lands.
        pb = psB.tile([128, NSUB, E], f32, tag="pb")
        for j in range(NSUB):
            for k in range(KH):
                nc.tensor.matmul(pb[:, j, :],
                                 lhsT=xgs[k // KHQ][:, k % KHQ,
                                                    j * 128:(j + 1) * 128],
                                 rhs=wt_sb[:, k, :],
                                 start=(k == 0), stop=(k == KH - 1))

        # ---- softmax over experts (free dim): exp + per-subtile sum on ACT
        sr = wk.tile([128, NSUB, E], f32, tag="sr")
        S_ = wk.tile([128, NSUB], f32, tag="S")
        for j in range(NSUB):
            nc.scalar.activation(sr[:, j, :], pb[:, j, :], func=Act.Exp,
                                 accum_out=S_[:, j:j + 1])
        R_ = wk.tile([128, NSUB], f32, tag="R")
        nc.vector.reciprocal(R_, S_)

        # ---- bias-adjusted scores z = exp*R + bias, and top-8
        z_ = wk.tile([128, NSUB, E], f32, tag="z")
        v_ = wk.tile([128, NSUB, K], f32, tag="v")
        ix = wk.tile([128, NSUB, K], u16, tag="ix")
        for j in range(NSUB):
            nc.vector.scalar_tensor_tensor(z_[:, j, :], sr[:, j, :],
                                           R_[:, j:j + 1], bias_sb,
                                           Alu.mult, Alu.add)
            nc.vector.max(out=v_[:, j, :], in_=z_[:, j, :])
            nc.vector.max_index(out=ix[:, j, :], in_max=v_[:, j, :],
                                in_values=z_[:, j, :])

        # ---- gather bias[idx] via one-hot arithmetic: s[idx] = v - bias[idx]
        ixf = wk.tile([128, NSUB * K], f32, tag="ixf")
        nc.vector.tensor_copy(ixf, ix.rearrange("p a b -> p (a b)"))
        oh = ohp.tile([128, NSUB * K, E], f32, tag="oh")
        nc.gpsimd.tensor_tensor(oh, ixf.unsqueeze(2).to_broadcast(
            [128, NSUB * K, E]), iota_b3, Alu.subtract)
        nc.vector.scalar_tensor_tensor(oh, oh, 0.0, bias_b3,
                                       Alu.is_equal, Alu.mult)
        bg = wk.tile([128, NSUB, K], f32, tag="bg")
        nc.vector.tensor_reduce(bg.rearrange("p a b -> p (a b)"), oh,
                                axis=Ax.X, op=Alu.add)
        sg = wk.tile([128, NSUB, K], f32, tag="sg")
        nc.vector.tensor_sub(sg, v_, bg)

        # ---- normalize top-8 weights; emit outputs
        S8 = wk.tile([128, NSUB], f32, tag="S8")
        nc.vector.tensor_reduce(S8, sg, axis=Ax.X, op=Alu.add)
        R8 = wk.tile([128, NSUB], f32, tag="R8")
        nc.vector.reciprocal(R8, S8)
        w_g = outp.tile([128, NSUB, K], f32, tag="w_g")
        for j in range(NSUB):
            nc.scalar.activation(w_g[:, j, :], sg[:, j, :], func=Act.Copy,
                                 scale=R8[:, j:j + 1])
        idx_g = outp.tile([128, NSUB, K], i32, tag="idx_g")
        nc.vector.tensor_copy(idx_g, ix)
        # per-group stores overlap with later groups' compute; issue on the
        # ACT DGE ring so they don't head-of-line-block sync-ring loads
        nc.scalar.dma_start(out=idx_out[:, g], in_=idx_g)
        nc.scalar.dma_start(out=w_out[:, g], in_=w_g)


_NC_CACHE = {}


def get_nc(t_core=T_CORE):
    if t_core not in _NC_CACHE:
        _NC_CACHE[t_core] = build_nc(t_core)
    return _NC_CACHE[t_core]


def _reorder(dev_out, t_core):
    # [128, G, NSUB, K] -> [t_core, K] with token = (g*NSUB+j)*128 + p
    return dev_out.transpose(1, 2, 0, 3).reshape(t_core, K)


def kernel(hidden_states, weight, expert_biases, top_k):
    from concourse.bass_utils import run_bass_kernel_spmd

    assert int(top_k) == K
    x2d = np.asarray(hidden_states, dtype=np.float32).reshape(-1, H)
    wt = np.ascontiguousarray(np.asarray(weight, dtype=np.float32).T)
    eb = np.ascontiguousarray(np.asarray(expert_biases, dtype=np.float32))

    nc = get_nc()
    in_maps = []
    for c in range(N_CORES):
        xc = np.ascontiguousarray(x2d[c * T_CORE:(c + 1) * T_CORE, :].T)
        in_maps.append({"xt": xc, "wt": wt, "eb": eb})
    res = run_bass_kernel_spmd(nc, in_maps, core_ids=list(range(N_CORES)))

    idxs, ws = [], []
    for c in range(N_CORES):
        r = res.results[c]
        idxs.append(_reorder(r["idx_out"], T_CORE))
        ws.append(_reorder(r["w_out"], T_CORE))
    return (np.concatenate(idxs, axis=0).astype(np.int32),
            np.concatenate(ws, axis=0).astype(np.float32))
